# revision 16
# baseline (speedup 1.0000x reference)
"""SVD++ prediction kernel for Trainium2 (8 NeuronCores, Bass/Tile).

Math (per batch element b with user u = x[b,0], item i = x[b,1]):
    y_sum  = sum_h Y[items_hist[u, h]]                  (H = 50)
    pred_b = mu + bu[u] + bi[i] + dot(P[u] + inv_sqrt[u] * y_sum, Q[i])

The HW bottleneck is the SWDGE/SDMA per-packet rate (~2.5 ns/packet
aggregate over the 4 queues, independent of packet size up to 512B), so the
kernel minimizes gather-packet count and keeps the rings continuously fed.

Sharding: the host sorts the batch by user id and splits it into 8
contiguous slices of 2048, so duplicate users cluster within a core. Each
core deduplicates its users (nd <= 2048 distinct, ~1400 typical for uniform
data) and computes y_sum once per DISTINCT user ("stage 1"), then gathers
those y_sum rows back per batch element ("stage 2"). Y-gather packets drop
from 102400 to ~nd*50 per core. The program is compiled per
n_chunks = ceil(max_core(nd)/128) (cached variants); dedup slots beyond nd
are padded with sentinel users (extra hist16 rows with spread fake
histories) so every gather index is valid -- no runtime counts needed.

Tables (replicated per core; all real ids < 20000):
    hist16 [20256,128] i16 : items_hist rows (+256 sentinel rows), 256B
    P_ext  [20000,128] f32 : [P row | bu | inv_sqrt | pad]  (512B rows)
    Q_ext  [20000,128] f32 : [Q row | bi | pad]             (512B rows)
    Y      [20000,128] bf16: Y rows padded to 256B

Per-pass pipeline (the steady-state program runs `passes` passes per
hardware-loop iteration with ping-pong buffers; each pass's aux gathers +
DVE index-fold run right after its compute, preparing the next iteration
while the following pass's Y-packet stream drains; the For_i back-edge
carries an all-engine barrier, so the body starts with Y desc-gen
immediately after it):
 1. hist-gather: 4096 slots (each dedup user's hist row gathered twice so
    the row for slot d = 128c+16j+q lands on partitions 32*(j%4)+q AND
    32*(j%4)+16+q, satisfying the DVE quadrant rule for the fold).
 2. fold: 32 quad-aligned 32-lane DVE copies build I16 (wrapped Y-index
    tensor; idx t of chunk c at partition t%16 within queue (c%4)'s
    32-partition window, col t//16; t = h*128 + d).
 3. per active chunk c: Y-gather of 6400 rows -> [128, 50, 128] bf16
    (queue c%4); DVE strided reduce over h -> ysum[:, c, :] (f32);
    DMA ysum chunk -> ysum_tab DRAM rows [128c, 128c+128).
 4. stage-2 gather: ysum_tab rows per batch element (via dedup inverse
    index) -> ysb [128, 16, 64] batch-major.
 5. P_ext/Q_ext gathers deliver pu, bu, inv_sqrt, qi, bi batch-major;
    batched DVE epilogue:
       pred = mu + bu + bi + dot(pu, qi) + inv_sqrt * dot(ysb, qi);
    one DMA writes out[128, 16] (pred of sorted element k = 128c+p at
    [p, c]); the host scatters back to original batch order.
"""
import os
import sys
import numpy as np
from contextlib import ExitStack

if "/opt/trn_rl_repo" not in sys.path:
    sys.path.insert(0, "/opt/trn_rl_repo")

import concourse.bacc as bacc
import concourse.tile as tile
import concourse.mybir as mybir
from concourse.bass_utils import run_bass_kernel_spmd

N_CORES = 8
B = 16384
BC = B // N_CORES          # per-core batch = 2048
C = BC // 128              # chunks of 128 dedup slots = 16
F = 64                     # factors
H = 50                     # history length
HP = 128                   # padded hist row (int16 -> 256B)
NI = 20000                 # addressable table rows (all ids < 20000)
NS = 256                   # sentinel hist rows for dedup padding

_PROGRAM_CACHE = {}
_N_CHUNKS = [C]            # set by build_in_maps from the actual data
LAST_RESULTS = None        # side-channel for test harness (profile access)

FP32 = mybir.dt.float32
BF16 = mybir.dt.bfloat16


def _build_program(reps=1, sim_safe=False, ybufs=8, n_chunks=None):
    if n_chunks is None:
        n_chunks = _N_CHUNKS[0]
    nc = bacc.Bacc("TRN2", target_bir_lowering=False, debug=False,
                   num_devices=N_CORES, num_swdge_queues=4)

    yt = nc.dram_tensor("Y", [NI, 128], BF16, kind="ExternalInput")
    histT = nc.dram_tensor("hist16", [NI + NS, HP], mybir.dt.int16,
                           kind="ExternalInput")
    pextT = nc.dram_tensor("P_ext", [NI, 128], FP32, kind="ExternalInput")
    qextT = nc.dram_tensor("Q_ext", [NI, 128], FP32, kind="ExternalInput")
    uhT = nc.dram_tensor("uh_wrap", [128, 2 * BC // 16], mybir.dt.int16,
                         kind="ExternalInput")
    uwT = nc.dram_tensor("u_wrap", [128, BC // 16], mybir.dt.int16,
                         kind="ExternalInput")
    iwT = nc.dram_tensor("i_wrap", [128, BC // 16], mybir.dt.int16,
                         kind="ExternalInput")
    ywT = nc.dram_tensor("ys_wrap", [128, BC // 16], mybir.dt.int16,
                         kind="ExternalInput")
    muT = nc.dram_tensor("mu", [128, 1], FP32, kind="ExternalInput")
    outT = nc.dram_tensor("out", [128, C], FP32, kind="ExternalOutput")

    two_phase = reps > 1
    assert reps == 1 or reps % 2 == 0
    passes = 4 if (reps % 4 == 0) else 2
    n_ph = 2 if two_phase else 1
    half = BC // 2

    ysumT = [nc.dram_tensor(f"ysum_tab{p}", [BC, F], FP32, kind="Internal")
             for p in range(n_ph)]

    with tile.TileContext(nc) as tc, ExitStack() as ctx:
        pool = ctx.enter_context(tc.tile_pool(name="main", bufs=1))
        gpool = ctx.enter_context(tc.tile_pool(name="yg", bufs=ybufs))

        uhw = pool.tile([128, 2 * BC // 16], mybir.dt.int16)
        nc.sync.dma_start(uhw[:], uhT[:])
        uw = pool.tile([128, BC // 16], mybir.dt.int16)
        nc.sync.dma_start(uw[:], uwT[:])
        iw = pool.tile([128, BC // 16], mybir.dt.int16)
        nc.sync.dma_start(iw[:], iwT[:])
        yw = pool.tile([128, BC // 16], mybir.dt.int16)
        nc.sync.dma_start(yw[:], ywT[:])
        muS = pool.tile([128, 1], FP32)
        nc.sync.dma_start(muS[:], muT[:])

        I16 = [pool.tile([128, C * 400], mybir.dt.int16, name=f"I16{p}",
                         tag=f"I16{p}") for p in range(n_ph)]
        hqs = [[pool.tile([128, 2 * C // 4, HP], mybir.dt.int16,
                          name=f"h{p}_{q}", tag=f"h{p}_{q}") for q in range(4)]
               for p in range(n_ph)]
        pg = [pool.tile([128, C, 128], FP32, name=f"pg{p}", tag=f"pg{p}")
              for p in range(n_ph)]
        qg = [pool.tile([128, C, 128], FP32, name=f"qg{p}", tag=f"qg{p}")
              for p in range(n_ph)]
        if sim_safe:
            for p in range(n_ph):
                nc.vector.memset(I16[p][:], 0)

        def nact(q):
            # active chunks (c < n_chunks) handled by quadrant q
            return n_chunks // 4 + (1 if q < n_chunks % 4 else 0)

        def gather_aux(p):
            # hist gathers (queue q, active chunk blocks only) + P/Q gathers
            for q in range(4):
                nidx = 2 * 128 * nact(q)
                nc.gpsimd.dma_gather(
                    hqs[p][q][:, 0:2 * nact(q), :], histT[:],
                    uhw[:, q * (BC // 32):q * (BC // 32) + nidx // 16],
                    nidx, nidx, HP,
                    single_packet=False, queue_num=q)
            nc.gpsimd.dma_gather(pg[p][:, 0:C // 2, :], pextT[:],
                                 uw[:, 0:half // 16],
                                 half, half, 128, single_packet=False,
                                 queue_num=1)
            nc.gpsimd.dma_gather(pg[p][:, C // 2:C, :], pextT[:],
                                 uw[:, half // 16:],
                                 half, half, 128, single_packet=False,
                                 queue_num=3)
            nc.gpsimd.dma_gather(qg[p][:, 0:C // 2, :], qextT[:],
                                 iw[:, 0:half // 16],
                                 half, half, 128, single_packet=False,
                                 queue_num=0)
            nc.gpsimd.dma_gather(qg[p][:, C // 2:C, :], qextT[:],
                                 iw[:, half // 16:],
                                 half, half, 128, single_packet=False,
                                 queue_num=2)

        def fold(p):
            I16v = I16[p][:].rearrange("p (c4 four h j) -> p c4 four h j",
                                       c4=C // 4, four=4, h=H, j=8)
            for q in range(4):
                na = nact(q)
                hv = hqs[p][q][:].rearrange("p (c4 two) e -> p c4 e two", two=2)
                for a in range(4):
                    nc.vector.tensor_copy(
                        I16v[32 * q:32 * q + 32, 0:na, q, :, a::4],
                        hv[32 * a:32 * a + 32, 0:na, 0:H, :])

        def compute_stage1(p):
            # stage 1: Y gathers (queue c%4) + segment reduce, active chunks
            ysum = pool.tile([128, C, F], FP32, tag=f"ysum{p}")
            for c in range(n_chunks):
                g = gpool.tile([128, H, 128], BF16, tag="yg")
                nc.gpsimd.dma_gather(
                    g[:], yt[:], I16[p][:, c * 400:(c + 1) * 400],
                    128 * H, 128 * H, 128,
                    single_packet=False, queue_num=c % 4)
                nc.vector.reduce_sum(
                    ysum[:, c, :],
                    g[:].rearrange("p h f -> p f h")[:, 0:F, :],
                    axis=mybir.AxisListType.X)
                nc.sync.dma_start(
                    ysumT[p][c * 128:(c + 1) * 128, :].rearrange(
                        "(one pp) f -> pp one f", pp=128),
                    ysum[:, c:c + 1, :])

        def compute_stage2(p):
            # stage 2: y_sum rows per batch element (dedup inverse index)
            ysb = pool.tile([128, C, F], FP32, tag=f"ysb{p}")
            qtr = BC // 4
            for k in range(4):
                nc.gpsimd.dma_gather(
                    ysb[:, 4 * k:4 * k + 4, :], ysumT[p][:],
                    yw[:, k * (qtr // 16):(k + 1) * (qtr // 16)],
                    qtr, qtr, F, single_packet=False, queue_num=k)

            # prediction epilogue, batched across chunks:
            #   pred = mu + bu + bi + dot(pu, qi) + inv_sqrt * dot(ysb, qi)
            tmp = pool.tile([128, C, F], FP32, tag=f"tmp{p}")
            d1 = pool.tile([128, C], FP32, tag=f"d1{p}")
            d2 = pool.tile([128, C], FP32, tag=f"d2{p}")
            nc.vector.tensor_mul(tmp[:], pg[p][:, :, 0:F], qg[p][:, :, 0:F])
            nc.vector.reduce_sum(d1[:], tmp[:], axis=mybir.AxisListType.X)
            nc.vector.tensor_mul(tmp[:], ysb[:], qg[p][:, :, 0:F])
            nc.vector.reduce_sum(d2[:], tmp[:], axis=mybir.AxisListType.X)
            nc.vector.tensor_mul(d2[:], d2[:], pg[p][:, :, F + 1])
            nc.vector.tensor_add(d1[:], d1[:], d2[:])
            nc.vector.tensor_add(d1[:], d1[:], pg[p][:, :, F])
            nc.vector.tensor_add(d1[:], d1[:], qg[p][:, :, F])
            ot = pool.tile([128, C], FP32, tag=f"ot{p}")
            nc.vector.tensor_scalar_add(ot[:, :], d1[:, :], muS[:, 0:1])
            nc.sync.dma_start(outT[:, :], ot[:, :])

        if not two_phase:
            gather_aux(0)
            fold(0)
            compute_stage1(0)
            compute_stage2(0)
        else:
            gather_aux(0)
            fold(0)
            gather_aux(1)
            fold(1)
            with tc.For_i(0, reps // passes, 1):
                for p_ in range(passes):
                    compute_stage1(p_ % 2)
                    gather_aux(p_ % 2)
                    compute_stage2(p_ % 2)
                    fold(p_ % 2)

    nc.compile()
    return nc


def _wrap16(v, n):
    # idx t read from [t%16, t//16]; replicate the 16-partition block x8
    w = np.ascontiguousarray(v.astype(np.int16).reshape(n // 16, 16).T)
    return np.tile(w, (8, 1))


def _sort_order(x):
    return np.argsort(np.asarray(x)[:, 0], kind="stable")


def build_in_maps(inputs):
    """Host-side shard/prep: per-core input dicts for run_bass_kernel_spmd."""
    x = np.asarray(inputs["x"])
    items_hist = np.asarray(inputs["items_hist"])
    P = np.asarray(inputs["P"], np.float32)
    Q = np.asarray(inputs["Q"], np.float32)
    bu = np.asarray(inputs["bu"], np.float32)
    bi = np.asarray(inputs["bi"], np.float32)
    Y = np.asarray(inputs["Y"], np.float32)
    inv_sqrt = np.asarray(inputs["inv_sqrt"], np.float32)
    mu = np.float32(np.asarray(inputs["mu"]))

    import ml_dtypes
    # shared table prep (all referenced ids are < NI)
    hist16 = np.zeros((NI + NS, HP), np.int16)
    hist16[:NI, :H] = items_hist[:NI].astype(np.int16)
    # sentinel rows: spread fake histories (avoid hot-banking one Y row)
    k = np.arange(NS)[:, None]
    h = np.arange(H)[None, :]
    hist16[NI:, :H] = ((k * 787 + h * 397) % NI).astype(np.int16)
    P_ext = np.zeros((NI, 128), np.float32)
    P_ext[:, :F] = P[:NI]
    P_ext[:, F] = bu[:NI]
    P_ext[:, F + 1] = inv_sqrt[:NI]
    Q_ext = np.zeros((NI, 128), np.float32)
    Q_ext[:, :F] = Q[:NI]
    Q_ext[:, F] = bi[:NI]
    Yc = np.zeros((NI, 128), ml_dtypes.bfloat16)
    Yc[:, :F] = Y[:NI].astype(ml_dtypes.bfloat16)
    mu_arr = np.full((128, 1), mu, np.float32)

    # hist-gather slot map: per-quad gathers; quad g covers chunks c with
    # c%4 == g, local slot i1 = q + 32*(j%4) + 128*(2*(c//4) + j//4)
    dd = np.arange(BC)
    qq, jj, cc = dd % 16, (dd % 128) // 16, dd // 128
    i1_base = (cc % 4) * (BC // 2) + qq + 32 * (jj % 4) + 128 * (2 * (cc // 4) + jj // 4)

    order = _sort_order(x)
    in_maps = []
    nds = []
    for core in range(N_CORES):
        sel = order[core * BC:(core + 1) * BC]
        su = x[sel, 0].astype(np.int64)
        si = x[sel, 1].astype(np.int16)
        D, inv = np.unique(su, return_inverse=True)
        nd = len(D)
        nds.append(nd)
        Dp = np.empty(BC, np.int16)
        Dp[:nd] = D.astype(np.int16)
        pad = np.arange(nd, BC)
        Dp[nd:] = (NI + (pad % NS)).astype(np.int16)
        hist_slots = np.zeros(2 * BC, np.int16)
        hist_slots[i1_base] = Dp
        hist_slots[i1_base + 16] = Dp
        in_maps.append({
            "Y": Yc, "hist16": hist16, "P_ext": P_ext, "Q_ext": Q_ext,
            "uh_wrap": _wrap16(hist_slots, 2 * BC),
            "u_wrap": _wrap16(su.astype(np.int16), BC),
            "i_wrap": _wrap16(si, BC),
            "ys_wrap": _wrap16(inv.astype(np.int16), BC),
            "mu": mu_arr,
        })

    _N_CHUNKS[0] = max(1, -(-max(nds) // 128))
    return in_maps


def kernel(x, items_hist, P, Q, bu, bi, Y, inv_sqrt, mu):
    global LAST_RESULTS
    in_maps = build_in_maps(dict(x=x, items_hist=items_hist, P=P, Q=Q, bu=bu,
                                 bi=bi, Y=Y, inv_sqrt=inv_sqrt, mu=mu))
    n_chunks = _N_CHUNKS[0]
    if n_chunks not in _PROGRAM_CACHE:
        _PROGRAM_CACHE[n_chunks] = _build_program(n_chunks=n_chunks)
    nc = _PROGRAM_CACHE[n_chunks]

    res = run_bass_kernel_spmd(nc, in_maps, list(range(N_CORES)))
    LAST_RESULTS = res

    order = _sort_order(x)
    pred = np.empty(B, np.float32)
    for core in range(N_CORES):
        o = np.asarray(res.results[core]["out"])    # [128, C]; k = 128c + p
        pred[order[core * BC:(core + 1) * BC]] = o.T.reshape(-1)
    return pred


# revision 18
# speedup vs baseline: 1.1609x; 1.1609x over previous
"""SVD++ prediction kernel for Trainium2 (8 NeuronCores, Bass/Tile).

Math (per batch element b with user u = x[b,0], item i = x[b,1]):
    y_sum  = sum_h Y[items_hist[u, h]]                  (H = 50)
    pred_b = mu + bu[u] + bi[i] + dot(P[u] + inv_sqrt[u] * y_sum, Q[i])

The HW bottleneck is the SWDGE/SDMA per-packet rate (~2.5 ns/packet
aggregate over the 4 queues, independent of packet size up to 512B), so the
kernel minimizes gather-packet count and keeps the rings continuously fed.

Sharding: the host sorts the batch by user id and splits it into 8
contiguous slices of 2048, so duplicate users cluster within a core. Each
core deduplicates its users (nd <= 2048 distinct, ~1400 typical for uniform
data) and computes y_sum once per DISTINCT user ("stage 1"), then gathers
those y_sum rows back per batch element ("stage 2"). Y-gather packets drop
from 102400 to ~nd*50 per core. The program is compiled per
n_chunks = ceil(max_core(nd)/128) (cached variants); dedup slots beyond nd
are padded with sentinel users (extra hist16 rows with spread fake
histories) so every gather index is valid -- no runtime counts needed.

Tables (replicated per core; all real ids < 20000):
    hist16 [20256,128] i16 : items_hist rows (+256 sentinel rows), 256B
    P_ext  [20000,128] f32 : [P row | bu | inv_sqrt | pad]  (512B rows)
    Q_ext  [20000,128] f32 : [Q row | bi | pad]             (512B rows)
    Y      [20000,128] bf16: Y rows padded to 256B

Per-pass pipeline (the steady-state program runs `passes` passes per
hardware-loop iteration with ping-pong buffers; each pass's aux gathers +
DVE index-fold run right after its compute, preparing the next iteration
while the following pass's Y-packet stream drains; the For_i back-edge
carries an all-engine barrier, so the body starts with Y desc-gen
immediately after it):
 1. hist-gather: 4096 slots (each dedup user's hist row gathered twice so
    the row for slot d = 128c+16j+q lands on partitions 32*(j%4)+q AND
    32*(j%4)+16+q, satisfying the DVE quadrant rule for the fold).
 2. fold: 32 quad-aligned 32-lane DVE copies build I16 (wrapped Y-index
    tensor; idx t of chunk c at partition t%16 within queue (c%4)'s
    32-partition window, col t//16; t = h*128 + d).
 3. per active chunk c: Y-gather of 6400 rows -> [128, 50, 128] bf16
    (queue c%4); DVE strided reduce over h -> ysum[:, c, :] (f32);
    DMA ysum chunk -> ysum_tab DRAM rows [128c, 128c+128).
 4. stage-2 gather: ysum_tab rows per batch element (via dedup inverse
    index) -> ysb [128, 16, 64] batch-major.
 5. P_ext/Q_ext gathers deliver pu, bu, inv_sqrt, qi, bi batch-major;
    batched DVE epilogue:
       pred = mu + bu + bi + dot(pu, qi) + inv_sqrt * dot(ysb, qi);
    one DMA writes out[128, 16] (pred of sorted element k = 128c+p at
    [p, c]); the host scatters back to original batch order.
"""
import os
import sys
import numpy as np
from contextlib import ExitStack

if "/opt/trn_rl_repo" not in sys.path:
    sys.path.insert(0, "/opt/trn_rl_repo")

import concourse.bacc as bacc
import concourse.tile as tile
import concourse.mybir as mybir
from concourse.bass_utils import run_bass_kernel_spmd

N_CORES = 8
B = 16384
BC = B // N_CORES          # per-core batch = 2048
C = BC // 128              # chunks of 128 dedup slots = 16
F = 64                     # factors
H = 50                     # history length
HP = 128                   # padded hist row (int16 -> 256B)
NI = 20000                 # addressable table rows (all ids < 20000)
NS = 256                   # sentinel hist rows for dedup padding

_PROGRAM_CACHE = {}
_N_CHUNKS = [C]            # set by build_in_maps from the actual data
LAST_RESULTS = None        # side-channel for test harness (profile access)

FP32 = mybir.dt.float32
BF16 = mybir.dt.bfloat16


def _build_program(reps=1, sim_safe=False, ybufs=8, n_chunks=None):
    if n_chunks is None:
        n_chunks = _N_CHUNKS[0]
    nc = bacc.Bacc("TRN2", target_bir_lowering=False, debug=False,
                   num_devices=N_CORES, num_swdge_queues=4)

    yt = nc.dram_tensor("Y", [NI, 128], BF16, kind="ExternalInput")
    histT = nc.dram_tensor("hist16", [NI + NS, HP], mybir.dt.int16,
                           kind="ExternalInput")
    pextT = nc.dram_tensor("P_ext", [NI, 128], FP32, kind="ExternalInput")
    qextT = nc.dram_tensor("Q_ext", [NI, 128], FP32, kind="ExternalInput")
    uhT = nc.dram_tensor("uh_wrap", [128, 2 * BC // 16], mybir.dt.int16,
                         kind="ExternalInput")
    uwT = nc.dram_tensor("u_wrap", [128, BC // 16], mybir.dt.int16,
                         kind="ExternalInput")
    iwT = nc.dram_tensor("i_wrap", [128, BC // 16], mybir.dt.int16,
                         kind="ExternalInput")
    ywT = nc.dram_tensor("ys_wrap", [128, BC // 16], mybir.dt.int16,
                         kind="ExternalInput")
    muT = nc.dram_tensor("mu", [128, 1], FP32, kind="ExternalInput")
    outT = nc.dram_tensor("out", [128, C], FP32, kind="ExternalOutput")

    two_phase = reps > 1
    assert reps == 1 or reps % 2 == 0
    passes = 4 if (reps % 4 == 0) else 2
    n_ph = 2 if two_phase else 1
    half = BC // 2

    ysumT = [nc.dram_tensor(f"ysum_tab{p}", [BC, F], FP32, kind="Internal")
             for p in range(n_ph)]

    with tile.TileContext(nc) as tc, ExitStack() as ctx:
        pool = ctx.enter_context(tc.tile_pool(name="main", bufs=1))
        gpool = ctx.enter_context(tc.tile_pool(name="yg", bufs=ybufs))

        uhw = pool.tile([128, 2 * BC // 16], mybir.dt.int16)
        nc.sync.dma_start(uhw[:], uhT[:])
        uw = pool.tile([128, BC // 16], mybir.dt.int16)
        nc.sync.dma_start(uw[:], uwT[:])
        iw = pool.tile([128, BC // 16], mybir.dt.int16)
        nc.sync.dma_start(iw[:], iwT[:])
        yw = pool.tile([128, BC // 16], mybir.dt.int16)
        nc.sync.dma_start(yw[:], ywT[:])
        muS = pool.tile([128, 1], FP32)
        nc.sync.dma_start(muS[:], muT[:])

        I16 = [pool.tile([128, C * 400], mybir.dt.int16, name=f"I16{p}",
                         tag=f"I16{p}") for p in range(n_ph)]
        hqs = [[pool.tile([128, 2 * C // 4, HP], mybir.dt.int16,
                          name=f"h{p}_{q}", tag=f"h{p}_{q}") for q in range(4)]
               for p in range(n_ph)]
        pg = [pool.tile([128, C, 128], FP32, name=f"pg{p}", tag=f"pg{p}")
              for p in range(n_ph)]
        qg = [pool.tile([128, C, 128], FP32, name=f"qg{p}", tag=f"qg{p}")
              for p in range(n_ph)]
        if sim_safe:
            for p in range(n_ph):
                nc.vector.memset(I16[p][:], 0)

        def nact(q):
            # active chunks (c < n_chunks) handled by quadrant q
            return n_chunks // 4 + (1 if q < n_chunks % 4 else 0)

        def gather_aux(p):
            # hist gathers (queue q, active chunk blocks only) + P/Q gathers
            for q in range(4):
                nidx = 2 * 128 * nact(q)
                nc.gpsimd.dma_gather(
                    hqs[p][q][:, 0:2 * nact(q), :], histT[:],
                    uhw[:, q * (BC // 32):q * (BC // 32) + nidx // 16],
                    nidx, nidx, HP,
                    single_packet=False, queue_num=q)
            nc.gpsimd.dma_gather(pg[p][:, 0:C // 2, :], pextT[:],
                                 uw[:, 0:half // 16],
                                 half, half, 128, single_packet=False,
                                 queue_num=1)
            nc.gpsimd.dma_gather(pg[p][:, C // 2:C, :], pextT[:],
                                 uw[:, half // 16:],
                                 half, half, 128, single_packet=False,
                                 queue_num=3)
            nc.gpsimd.dma_gather(qg[p][:, 0:C // 2, :], qextT[:],
                                 iw[:, 0:half // 16],
                                 half, half, 128, single_packet=False,
                                 queue_num=0)
            nc.gpsimd.dma_gather(qg[p][:, C // 2:C, :], qextT[:],
                                 iw[:, half // 16:],
                                 half, half, 128, single_packet=False,
                                 queue_num=2)

        def fold(p):
            I16v = I16[p][:].rearrange("p (c4 four h j) -> p c4 four h j",
                                       c4=C // 4, four=4, h=H, j=8)
            for q in range(4):
                na = nact(q)
                hv = hqs[p][q][:].rearrange("p (c4 two) e -> p c4 e two", two=2)
                for a in range(4):
                    nc.vector.tensor_copy(
                        I16v[32 * q:32 * q + 32, 0:na, q, :, a::4],
                        hv[32 * a:32 * a + 32, 0:na, 0:H, :])

        def compute_stage1(p):
            # stage 1: Y gathers (queue c%4) + segment reduce, active chunks
            ysum = pool.tile([128, C, F], FP32, tag=f"ysum{p}")
            for c in range(n_chunks):
                g = gpool.tile([128, H, 128], BF16, tag="yg")
                nc.gpsimd.dma_gather(
                    g[:], yt[:], I16[p][:, c * 400:(c + 1) * 400],
                    128 * H, 128 * H, 128,
                    single_packet=False, queue_num=c % 4)
                nc.vector.reduce_sum(
                    ysum[:, c, :],
                    g[:].rearrange("p h f -> p f h")[:, 0:F, :],
                    axis=mybir.AxisListType.X)
                nc.sync.dma_start(
                    ysumT[p][c * 128:(c + 1) * 128, :].rearrange(
                        "(one pp) f -> pp one f", pp=128),
                    ysum[:, c:c + 1, :])

        def compute_stage2(p):
            # stage 2: y_sum rows per batch element (dedup inverse index)
            ysb = pool.tile([128, C, F], FP32, tag=f"ysb{p}")
            qtr = BC // 4
            for k in range(4):
                nc.gpsimd.dma_gather(
                    ysb[:, 4 * k:4 * k + 4, :], ysumT[p][:],
                    yw[:, k * (qtr // 16):(k + 1) * (qtr // 16)],
                    qtr, qtr, F, single_packet=False, queue_num=k)

            # prediction epilogue, batched across chunks:
            #   pred = mu + bu + bi + dot(pu, qi) + inv_sqrt * dot(ysb, qi)
            tmp = pool.tile([128, C, F], FP32, tag=f"tmp{p}")
            d1 = pool.tile([128, C], FP32, tag=f"d1{p}")
            d2 = pool.tile([128, C], FP32, tag=f"d2{p}")
            nc.vector.tensor_mul(tmp[:], pg[p][:, :, 0:F], qg[p][:, :, 0:F])
            nc.vector.reduce_sum(d1[:], tmp[:], axis=mybir.AxisListType.X)
            nc.vector.tensor_mul(tmp[:], ysb[:], qg[p][:, :, 0:F])
            nc.vector.reduce_sum(d2[:], tmp[:], axis=mybir.AxisListType.X)
            nc.vector.tensor_mul(d2[:], d2[:], pg[p][:, :, F + 1])
            nc.vector.tensor_add(d1[:], d1[:], d2[:])
            nc.vector.tensor_add(d1[:], d1[:], pg[p][:, :, F])
            nc.vector.tensor_add(d1[:], d1[:], qg[p][:, :, F])
            ot = pool.tile([128, C], FP32, tag=f"ot{p}")
            nc.vector.tensor_scalar_add(ot[:, :], d1[:, :], muS[:, 0:1])
            nc.sync.dma_start(outT[:, :], ot[:, :])

        if not two_phase:
            gather_aux(0)
            fold(0)
            compute_stage1(0)
            compute_stage2(0)
        else:
            gather_aux(0)
            fold(0)
            gather_aux(1)
            fold(1)
            with tc.For_i(0, reps // passes, 1):
                for p_ in range(passes):
                    compute_stage1(p_ % 2)
                    gather_aux(p_ % 2)
                    compute_stage2(p_ % 2)
                    fold(p_ % 2)

    nc.compile()
    return nc


def _wrap16(v, n):
    # idx t read from [t%16, t//16]; replicate the 16-partition block x8
    w = np.ascontiguousarray(v.astype(np.int16).reshape(n // 16, 16).T)
    return np.tile(w, (8, 1))


def _sort_order(x):
    return np.argsort(np.asarray(x)[:, 0], kind="stable")


def build_in_maps(inputs):
    """Host-side shard/prep: per-core input dicts for run_bass_kernel_spmd."""
    x = np.asarray(inputs["x"])
    items_hist = np.asarray(inputs["items_hist"])
    P = np.asarray(inputs["P"], np.float32)
    Q = np.asarray(inputs["Q"], np.float32)
    bu = np.asarray(inputs["bu"], np.float32)
    bi = np.asarray(inputs["bi"], np.float32)
    Y = np.asarray(inputs["Y"], np.float32)
    inv_sqrt = np.asarray(inputs["inv_sqrt"], np.float32)
    mu = np.float32(np.asarray(inputs["mu"]))

    import ml_dtypes
    # shared table prep (all referenced ids are < NI)
    hist16 = np.zeros((NI + NS, HP), np.int16)
    hist16[:NI, :H] = items_hist[:NI].astype(np.int16)
    # sentinel rows: spread fake histories (avoid hot-banking one Y row)
    k = np.arange(NS)[:, None]
    h = np.arange(H)[None, :]
    hist16[NI:, :H] = ((k * 787 + h * 397) % NI).astype(np.int16)
    P_ext = np.zeros((NI, 128), np.float32)
    P_ext[:, :F] = P[:NI]
    P_ext[:, F] = bu[:NI]
    P_ext[:, F + 1] = inv_sqrt[:NI]
    Q_ext = np.zeros((NI, 128), np.float32)
    Q_ext[:, :F] = Q[:NI]
    Q_ext[:, F] = bi[:NI]
    Yc = np.zeros((NI, 128), ml_dtypes.bfloat16)
    Yc[:, :F] = Y[:NI].astype(ml_dtypes.bfloat16)
    mu_arr = np.full((128, 1), mu, np.float32)

    # hist-gather slot map: per-quad gathers; quad g covers chunks c with
    # c%4 == g, local slot i1 = q + 32*(j%4) + 128*(2*(c//4) + j//4)
    dd = np.arange(BC)
    qq, jj, cc = dd % 16, (dd % 128) // 16, dd // 128
    i1_base = (cc % 4) * (BC // 2) + qq + 32 * (jj % 4) + 128 * (2 * (cc // 4) + jj // 4)

    order = _sort_order(x)
    in_maps = []
    nds = []
    for core in range(N_CORES):
        sel = order[core * BC:(core + 1) * BC]
        su = x[sel, 0].astype(np.int64)
        si = x[sel, 1].astype(np.int16)
        D, inv = np.unique(su, return_inverse=True)
        nd = len(D)
        nds.append(nd)
        Dp = np.empty(BC, np.int16)
        Dp[:nd] = D.astype(np.int16)
        pad = np.arange(nd, BC)
        Dp[nd:] = (NI + (pad % NS)).astype(np.int16)
        hist_slots = np.zeros(2 * BC, np.int16)
        hist_slots[i1_base] = Dp
        hist_slots[i1_base + 16] = Dp
        in_maps.append({
            "Y": Yc, "hist16": hist16, "P_ext": P_ext, "Q_ext": Q_ext,
            "uh_wrap": _wrap16(hist_slots, 2 * BC),
            "u_wrap": _wrap16(su.astype(np.int16), BC),
            "i_wrap": _wrap16(si, BC),
            "ys_wrap": _wrap16(inv.astype(np.int16), BC),
            "mu": mu_arr,
        })

    _N_CHUNKS[0] = max(1, -(-max(nds) // 128))
    return in_maps


def kernel(x, items_hist, P, Q, bu, bi, Y, inv_sqrt, mu):
    global LAST_RESULTS
    in_maps = build_in_maps(dict(x=x, items_hist=items_hist, P=P, Q=Q, bu=bu,
                                 bi=bi, Y=Y, inv_sqrt=inv_sqrt, mu=mu))
    n_chunks = _N_CHUNKS[0]
    if n_chunks not in _PROGRAM_CACHE:
        _PROGRAM_CACHE[n_chunks] = _build_program(n_chunks=n_chunks)
    nc = _PROGRAM_CACHE[n_chunks]

    res = run_bass_kernel_spmd(nc, in_maps, list(range(N_CORES)))
    LAST_RESULTS = res

    order = _sort_order(x)
    pred = np.empty(B, np.float32)
    for core in range(N_CORES):
        o = np.asarray(res.results[core]["out"])    # [128, C]; k = 128c + p
        pred[order[core * BC:(core + 1) * BC]] = o.T.reshape(-1)
    return pred
